# revision 12
# baseline (speedup 1.0000x reference)
"""Multi-head attention kernel for Trainium2 (Bass/Tile), 8-core data parallel.

Problem: B=32, N=1024, D=512, H=8 (per-head dim = D = 512).
  kh = k @ Wk[h].T + bk ; vh = v @ Wv[h].T + bv ; qh = q @ Wq[h].T + bq
  S = qh @ kh.T / sqrt(D); P = softmax(S); out_h = P @ vh
  rep = concat_interleaved(out_h) @ Wo.T + bo

Algebraic fusion (host precompute, float64):
  S = qh @ kh.T = (q @ M_h + c_h) @ k.T  with  M_h = Wq[h].T @ Wk[h],
  c_h = bq[h] @ Wk[h]   (bk and the bq.bk cross terms are per-query
  constants -> softmax-invariant -> dropped).
  P @ vh @ Wo_h.T = (P @ v) @ G_h  with  G_h = Wv[h].T @ Wo_h.T  and the
  bv term folds into bo (rows of P sum to 1):
  bo_eff = bo + sum_h bv[h] @ Wo_h.T.
This removes the K and V projections entirely: 25% fewer PE rows and
half the weight DMA vs computing kh/vh explicitly.

Sharding: batch data-parallel, 4 batches per core. All math per (b, h) is
done in "transposed" (feature-on-partition) orientation, except v which is
used in natural [n, d] layout as the PV lhsT:
  qtT[e,i]  = matmul(lhsT=M_h, rhs=qT)          (+ c_h during PSUM eviction)
  ST[j,i]   = matmul(lhsT=kT,  rhs=qtT)
  E[j,i]    = exp(ST/sqrt(D))                   (no max-subtract: scores ~N(0,1))
  denom     = onesT @ [sum_j E_j]  (chunk tiles pre-summed on DVE; the
              all-ones matmul does the cross-partition key reduction)
  PvT[d,i]  = matmul(lhsT=v,   rhs=E) * (1/denom)  (rescale on eviction)
  repT[eo,i]+= matmul(lhsT=G_h, rhs=PvT)        (accumulate heads in SBUF)
  out = repT + bo_eff

Matmul operands use float32r (full PE rate at free-dim>=256, ~1.5e-4 rel err).
"""
import math
from contextlib import ExitStack

import numpy as np

import concourse.bacc as bacc
import concourse.mybir as mybir
import concourse.tile as tile
from concourse.bass_utils import run_bass_kernel_spmd

dt = mybir.dt
P = 128

B, N, D, H = 32, 1024, 512, 8
NCORES = 8
BLOC = B // NCORES

FD = 512           # matmul free-dim / PSUM bank width (f32)
SCALE = 1.0 / math.sqrt(D)


class _Ctx:
    pass


def build_core_program(bloc=BLOC, n=N, d=D, h_cnt=H, reps=1):
    """Bass program for one core: bloc batches, full heads."""
    c = _Ctx()
    c.DC = d // P        # d-partition chunks (4)
    c.EC = d // P        # output-feature chunks (4)
    c.IC = n // FD       # query free-dim chunks (2)
    c.JC8 = n // P       # key partition chunks (8)
    c.n, c.d, c.h_cnt = n, d, h_cnt

    nc = bacc.Bacc("TRN2", target_bir_lowering=False, debug=False)
    c.nc = nc
    c.pending_g = None
    c.pending_g_next = 0
    c.pending_fin = None

    f32, f32r = dt.float32, dt.float32r
    c.f32, c.f32r = f32, f32r
    c.qT = nc.dram_tensor("qT", [bloc, d, n], f32r, kind="ExternalInput")
    c.kT = nc.dram_tensor("kT", [bloc, d, n], f32r, kind="ExternalInput")
    c.vN = nc.dram_tensor("vN", [bloc, n, d], f32r, kind="ExternalInput")
    c.MT = nc.dram_tensor("MT", [h_cnt, d, d], f32r, kind="ExternalInput")
    c.GT = nc.dram_tensor("GT", [h_cnt, d, d], f32r, kind="ExternalInput")
    c.c_d = nc.dram_tensor("c_d", [P, h_cnt * c.EC], f32, kind="ExternalInput")
    c.bo_d = nc.dram_tensor("bo_d", [P, c.EC], f32, kind="ExternalInput")
    c.ones_d = nc.dram_tensor("ones_d", [P, P], f32r, kind="ExternalInput")
    c.outT = nc.dram_tensor("outT", [bloc, d, n], f32, kind="ExternalOutput")

    c.AF = mybir.ActivationFunctionType

    with tile.TileContext(nc) as tc, ExitStack() as es:
        ep = es.enter_context
        c.const = ep(tc.tile_pool(name="const", bufs=1))
        c.acts = ep(tc.tile_pool(name="acts", bufs=1))
        c.mwp = ep(tc.tile_pool(name="mw", bufs=2))
        c.gwp = ep(tc.tile_pool(name="gw", bufs=2))
        c.projp = ep(tc.tile_pool(name="proj", bufs=1))
        c.esbp = ep(tc.tile_pool(name="esb", bufs=10))
        c.esump = ep(tc.tile_pool(name="esum", bufs=2))
        c.outnp = ep(tc.tile_pool(name="outn", bufs=2))
        c.recipp = ep(tc.tile_pool(name="recip", bufs=2))
        c.repp = ep(tc.tile_pool(name="rep", bufs=2))
        c.ps_s = ep(tc.tile_pool(name="ps_s", bufs=3, space="PSUM"))
        c.ps_pv = ep(tc.tile_pool(name="ps_pv", bufs=4, space="PSUM"))
        c.ps_d = ep(tc.tile_pool(name="ps_d", bufs=1, space="PSUM"))

        c.ones = c.const.tile([P, P], f32r, name="ones")
        nc.sync.dma_start(c.ones[:], c.ones_d[:])
        c.c_sb = c.const.tile([P, h_cnt * c.EC], f32, name="c_sb")
        nc.sync.dma_start(c.c_sb[:], c.c_d[:])
        c.bo_sb = c.const.tile([P, c.EC], f32, name="bo_sb")
        nc.sync.dma_start(c.bo_sb[:], c.bo_d[:])

        seq = [b for _ in range(reps) for b in range(bloc)]
        tiles = _issue_batch_dmas(c, seq[0])
        for i, b in enumerate(seq):
            nxt = seq[i + 1] if i + 1 < len(seq) else None
            tiles = _emit_batch(c, b, tiles, nxt)
        _flush_g(c)
        _finalize_batch(c)

    nc.compile()
    return nc


def _issue_batch_dmas(c, b):
    """Allocate + DMA batch b's activations and head-0 weights.

    The HWDGE queue is serial, so issue in first-use order: mw, then q
    chunks (first projection group becomes runnable quickly), then k, v, gw.
    Called at the END of the previous batch's compute emission so these
    transfers queue AHEAD of the previous batch's output DMA.
    """
    nc = c.nc
    qt = c.acts.tile([P, c.DC, c.n], c.f32r, name="qt")
    kt = c.acts.tile([P, c.DC, c.n], c.f32r, name="kt")
    vt = c.acts.tile([P, c.JC8, FD], c.f32r, name="vt")
    mw = c.mwp.tile([P, c.DC, c.d], c.f32r, name="mw")
    gw = c.gwp.tile([P, c.DC, c.d], c.f32r, name="gw")
    # interleave mw/qt per contraction chunk so the first projection
    # group's dc-th matmul fires as soon as its own chunks land
    for dcx in range(c.DC):
        nc.sync.dma_start(mw[:, dcx, :], c.MT[0, dcx * P:(dcx + 1) * P, :])
        nc.sync.dma_start(qt[:, dcx, :], c.qT[b, dcx * P:(dcx + 1) * P, :])
    for dcx in range(c.DC):
        nc.sync.dma_start(kt[:, dcx, :], c.kT[b, dcx * P:(dcx + 1) * P, :])
    nc.sync.dma_start(vt[:], c.vN[b].rearrange("(c p) d -> p c d", p=P))
    nc.sync.dma_start(gw[:], c.GT[0].rearrange("(c p) e -> p c e", p=P))
    return qt, kt, vt, mw, gw


def _finalize_batch(c):
    """bo bias-add + output DMA for a finished batch (deferred so it runs
    in the shadow of the next batch's q-projection)."""
    if c.pending_fin is None:
        return
    nc = c.nc
    b, repT = c.pending_fin
    c.pending_fin = None
    for ec in range(c.EC):
        nc.vector.tensor_scalar_add(
            repT[:, ec, :], repT[:, ec, :], c.bo_sb[:, ec:ec + 1])
        nc.sync.dma_start(
            c.outT[b, ec * P:(ec + 1) * P, :], repT[:, ec, :])


def _emit_batch(c, b, tiles, next_b):
    qt, kt, vt, mw, gw = tiles

    repT = c.repp.tile([P, c.EC, c.n], c.f32, name="repT")

    for h in range(c.h_cnt):
        _emit_head(c, h, qt, kt, vt, repT, (mw, gw) if h == 0 else None)

    # Queue the next batch's input DMAs ahead of this batch's output DMA,
    # and leave the last G groups + bias-add deferred into its q-phase.
    nxt = _issue_batch_dmas(c, next_b) if next_b is not None else None
    c.pending_fin = (b, repT)
    return nxt


def _issue_weight_dmas(c, h):
    nc = c.nc
    mw = c.mwp.tile([P, c.DC, c.d], c.f32r, name="mw")
    gw = c.gwp.tile([P, c.DC, c.d], c.f32r, name="gw")
    nc.sync.dma_start(mw[:], c.MT[h].rearrange("(c p) e -> p c e", p=P))
    nc.sync.dma_start(gw[:], c.GT[h].rearrange("(c p) e -> p c e", p=P))
    return mw, gw


def _emit_g_group(c, ec):
    """Emit one deferred output-projection group (+ repT accumulate)."""
    nc = c.nc
    h, ic, gw, outn, repT = c.pending_g
    i_sl = slice(ic * FD, (ic + 1) * FD)
    po = c.ps_s.tile([P, FD], c.f32, name="ps_s")
    for dc in range(c.DC):
        nc.tensor.matmul(
            po[:], gw[:, dc, ec * P:(ec + 1) * P], outn[:, dc, :],
            start=(dc == 0), stop=(dc == c.DC - 1))
    if h == 0:
        nc.vector.tensor_copy(repT[:, ec, i_sl], po[:])
    else:
        nc.vector.tensor_add(repT[:, ec, i_sl], repT[:, ec, i_sl], po[:])


def _flush_g(c, n_keep=0):
    while c.pending_g is not None and c.pending_g_next < c.EC - n_keep:
        _emit_g_group(c, c.pending_g_next)
        c.pending_g_next += 1
    if c.pending_g_next >= c.EC:
        c.pending_g = None
    if c.pending_g is None:
        _finalize_batch(c)


def _defer_g(c, h, ic, gw, outn, repT):
    _flush_g(c)
    c.pending_g = (h, ic, gw, outn, repT)
    c.pending_g_next = 0


def _maybe_g(c):
    """Emit the next pending G group, if any (one per call site)."""
    if c.pending_g is not None and c.pending_g_next < c.EC:
        _emit_g_group(c, c.pending_g_next)
        c.pending_g_next += 1
        if c.pending_g_next >= c.EC:
            c.pending_g = None
            _finalize_batch(c)


def _emit_head(c, h, qt, kt, vt, repT, w0=None):
    nc = c.nc
    DC, EC, IC = c.DC, c.EC, c.IC

    mw, gw = w0 if w0 is not None else _issue_weight_dmas(c, h)

    # ---- fused q projection: qtT[e,i] = M_h.T-free @ qT (+ c_h) ----
    # ic-outer so all evictions S(ic=0) needs are issued in the first half.
    # The previous chunk's deferred G groups interleave into this stream.
    qhT = c.projp.tile([P, EC, c.n], c.f32r, name="qhT")
    for ic in range(IC):
        for ec in range(EC):
            pq = c.ps_s.tile([P, FD], c.f32, name="ps_s")
            for dc in range(DC):
                nc.tensor.matmul(
                    pq[:], mw[:, dc, ec * P:(ec + 1) * P],
                    qt[:, dc, ic * FD:(ic + 1) * FD],
                    start=(dc == 0), stop=(dc == DC - 1))
            nc.scalar.activation(
                qhT[:, ec, ic * FD:(ic + 1) * FD], pq[:], c.AF.Identity,
                bias=c.c_sb[:, h * EC + ec:h * EC + ec + 1])
            if ic + ec >= 2:
                _maybe_g(c)

    # ---- attention + output projection, per query chunk ----
    for ic in range(IC):
        _emit_attention_chunk(c, h, ic, qhT, kt, vt, gw, repT)


def _emit_attention_chunk(c, h, ic, qhT, kt, vt, gw, repT):
    nc = c.nc
    DC, EC, JC8 = c.DC, c.EC, c.JC8
    i_sl = slice(ic * FD, (ic + 1) * FD)

    pv_ps = [c.ps_pv.tile([P, FD], c.f32, name="ps_pv") for _ in range(DC)]
    den_ps = c.ps_d.tile([P, FD], c.f32, name="ps_d")
    e_tiles = [None] * JC8

    # Software-pipelined with lag 2: PV(j-2) matmuls issue right after
    # S(j), so the PE never sits at a phase boundary waiting for the ACT
    # exp evictions. The PV accumulation groups (one per dc bank) stay
    # open across the whole chunk; interleaving groups across different
    # PSUM banks is legal. The softmax denominator needs a PARTITION
    # reduction (keys live on partitions), so chunk tiles are pre-summed
    # elementwise on DVE (j=0..6, in the shadow of S) and a single
    # 2-matmul ones-group reduces [esum, e7] late in the PV tail.
    esum = c.esump.tile([P, FD], c.f32r, name="esum")

    def issue_s(j):
        st = c.ps_s.tile([P, FD], c.f32, name="ps_s")
        for ec in range(EC):
            nc.tensor.matmul(
                st[:], kt[:, ec, j * P:(j + 1) * P], qhT[:, ec, i_sl],
                start=(ec == 0), stop=(ec == EC - 1))
        e_sb = c.esbp.tile([P, FD], c.f32r, name="e_sb")
        nc.scalar.activation(e_sb[:], st[:], c.AF.Exp, scale=SCALE)
        e_tiles[j] = e_sb
        if j == 1:
            nc.vector.tensor_add(esum[:], e_tiles[0][:], e_tiles[1][:])
        elif 1 < j < JC8 - 1:
            nc.vector.tensor_add(esum[:], esum[:], e_sb[:])

    def issue_pv(j):
        for dc in range(DC):
            nc.tensor.matmul(
                pv_ps[dc][:], vt[:, j, dc * P:(dc + 1) * P], e_tiles[j][:],
                start=(j == 0), stop=(j == JC8 - 1))

    issue_s(0)
    issue_s(1)
    for j in range(2, JC8):
        issue_s(j)
        issue_pv(j - 2)
        _maybe_g(c)
    issue_pv(JC8 - 2)
    nc.tensor.matmul(den_ps[:], c.ones[:], esum[:], start=True, stop=False)
    nc.tensor.matmul(den_ps[:], c.ones[:], e_tiles[JC8 - 1][:],
                     start=False, stop=True)
    recip = c.recipp.tile([P, FD], c.f32, name="recip")
    nc.vector.reciprocal(recip[:], den_ps[:])
    issue_pv(JC8 - 1)
    _flush_g(c)
    outn = c.outnp.tile([P, DC, FD], c.f32r, name="outn")
    for dc in range(DC):
        nc.vector.tensor_mul(outn[:, dc, :], pv_ps[dc][:], recip[:])

    # output projection is DEFERRED: its po groups interleave into the
    # next chunk's S phase (or the next head's q-projection), so the
    # serial recip/mul tail never gates the PE.
    _defer_g(c, h, ic, gw, outn, repT)


_CACHED_NC = None


def _get_nc():
    global _CACHED_NC
    if _CACHED_NC is None:
        _CACHED_NC = build_core_program()
    return _CACHED_NC


def _prep_in_maps(q, k, v, Wq, bq, Wk, bk, Wv, bv, Wo, bo):
    """Host-side fusion + layout prep + sharding. Returns per-core input maps."""
    f32, f64 = np.float32, np.float64
    qT = np.ascontiguousarray(
        q.reshape(NCORES, BLOC, N, D).transpose(0, 1, 3, 2)).astype(f32, copy=False)
    kT = np.ascontiguousarray(
        k.reshape(NCORES, BLOC, N, D).transpose(0, 1, 3, 2)).astype(f32, copy=False)
    vN = np.ascontiguousarray(v.reshape(NCORES, BLOC, N, D)).astype(f32, copy=False)

    # M_h = Wq[h].T @ Wk[h]  [d, e];  c_h = bq[h] @ Wk[h]  [e]
    M = np.einsum("hfd,hfe->hde", Wq.astype(f64), Wk.astype(f64)).astype(f32)
    cvec = np.einsum("hf,hfe->he", bq.astype(f64), Wk.astype(f64)).astype(f32)
    # Wo_h[eo, d] = Wo[eo, d*H + h];  G_h = Wv[h].T @ Wo_h.T  [f, eo]
    WoT = Wo.reshape(D, D, H).transpose(2, 1, 0).astype(f64)   # [h, d, eo]
    G = np.einsum("hdf,hde->hfe", Wv.astype(f64), WoT).astype(f32)
    # bo_eff = bo + sum_h bv[h] @ Wo_h.T
    bo_eff = bo.astype(f32) + np.einsum(
        "hd,hde->e", bv.astype(f64), WoT).astype(f32)

    # c_dev[p, h*EC + ec] = cvec[h, ec*128 + p]
    c_dev = np.ascontiguousarray(
        cvec.reshape(H, D // P, P).transpose(2, 0, 1).reshape(P, -1)).astype(f32)
    bo_dev = np.ascontiguousarray(bo_eff.reshape(D // P, P).T).astype(f32)
    ones = np.ones((P, P), f32)

    shared = dict(MT=M, GT=G, c_d=c_dev, bo_d=bo_dev, ones_d=ones)
    return [dict(qT=qT[c], kT=kT[c], vN=vN[c], **shared) for c in range(NCORES)]


def kernel(**inputs):
    nc = _get_nc()
    in_maps = _prep_in_maps(
        inputs["q"], inputs["k"], inputs["v"],
        inputs["Wq"], inputs["bq"], inputs["Wk"], inputs["bk"],
        inputs["Wv"], inputs["bv"], inputs["Wo"], inputs["bo"])
    res = run_bass_kernel_spmd(nc, in_maps, list(range(NCORES)))
    out = np.stack([res.results[c]["outT"] for c in range(NCORES)])  # [8,4,D,N]
    return np.ascontiguousarray(
        out.transpose(0, 1, 3, 2).reshape(B, N, D)).astype(np.float32)
